# revision 1
# baseline (speedup 1.0000x reference)
"""Decoder layer (attn + FFN + 2 layernorms) on 8 Trainium2 cores.

Sharding: core c handles batch b = c//4, query chunk i = c%4 (512 tokens).
Each core redundantly computes K/V for the full sequence (communication-free).
Causality is handled by rotating the key/value token order per core on the
host (self chunk first, then past, then future) so the mask structure is
uniform across cores: k-tiles 0-3 (the self chunk) get compile-time
triangular masks, the rest get a per-core additive bias (0 for past,
-1e30 for future) folded into the softmax exp. Softmax runs unnormalized
(no max subtraction; scores are O(+-8)) with the denominator taken from an
appended ones-column on V, and the division folded into the context copy.

K/V are computed and consumed chunk-by-chunk (fused with attention) so they
never need full SBUF residency; per-head context accumulates in SBUF across
chunks. The FFN intermediate (d_ff=4096) bounces through DRAM. Score
matmuls (contraction = head_dim 64) run as head PAIRS on disjoint PE
row-strips via tile_position for ~2x concurrency.

All matmuls run in float32r (TF32-like fast fp32 mode: 1 cycle/row at
free-dim >= 256 vs 4 cycles/row for exact fp32).
"""

import sys

sys.path.insert(0, "/opt/trn_rl_repo")

import numpy as np

D = 1024          # d_model
H = 16            # heads
HD = 64           # head dim
DFF = 4096
EPS = 1e-6
B, S = 2, 2048
QCH = 512         # query tokens per core
NCORES = 8
P = 128
NCH = S // QCH            # 4 chunks of k/v tokens
KT_TILES = S // P         # 16 k tiles
NDT = D // P              # 8 d_model tiles
NFT = DFF // P            # 32 d_ff tiles
QT_T = QCH // P           # 4 query token tiles
NEG = -1.0e30

_CACHE = {}


def _build(mm_dtype_name="float32r", debug=False):
    import concourse.bacc as bacc
    import concourse.mybir as mybir
    import concourse.tile as tile
    from concourse.masks import make_identity

    dt = mybir.dt
    MMDT = getattr(dt, mm_dtype_name)
    AF = mybir.ActivationFunctionType
    OP = mybir.AluOpType

    nc = bacc.Bacc("TRN2", target_bir_lowering=False, debug=False)

    # ---- I/O ----
    xb = nc.dram_tensor("xb", [S, D], dt.float32, kind="ExternalInput")
    kbias = nc.dram_tensor("kbias", [P, KT_TILES], dt.float32, kind="ExternalInput")
    Wq = nc.dram_tensor("Wq", [D, D], dt.float32, kind="ExternalInput")
    Wk = nc.dram_tensor("Wk", [D, D], dt.float32, kind="ExternalInput")
    Wv = nc.dram_tensor("Wv", [D, D], dt.float32, kind="ExternalInput")
    Wo = nc.dram_tensor("Wo", [D, D], dt.float32, kind="ExternalInput")
    W1 = nc.dram_tensor("W1", [D, DFF], dt.float32, kind="ExternalInput")
    W2 = nc.dram_tensor("W2", [DFF, D], dt.float32, kind="ExternalInput")
    bq = nc.dram_tensor("bq", [D], dt.float32, kind="ExternalInput")
    bk = nc.dram_tensor("bk", [D], dt.float32, kind="ExternalInput")
    bv = nc.dram_tensor("bv", [D], dt.float32, kind="ExternalInput")
    bo = nc.dram_tensor("bo", [D], dt.float32, kind="ExternalInput")
    b1 = nc.dram_tensor("b1", [DFF], dt.float32, kind="ExternalInput")
    b2 = nc.dram_tensor("b2", [D], dt.float32, kind="ExternalInput")
    g1 = nc.dram_tensor("g1", [D], dt.float32, kind="ExternalInput")
    be1 = nc.dram_tensor("be1", [D], dt.float32, kind="ExternalInput")
    g2 = nc.dram_tensor("g2", [D], dt.float32, kind="ExternalInput")
    be2 = nc.dram_tensor("be2", [D], dt.float32, kind="ExternalInput")
    out = nc.dram_tensor("out", [QCH, D], dt.float32, kind="ExternalOutput")
    dbg = {}
    if debug:
        for nm, shp in [("dbg_xqT", [P, NDT, QCH]), ("dbg_QT", [P, NDT, QCH]),
                        ("dbg_kt", [P, NDT, QCH]), ("dbg_v", [P, QT_T, H, HD + 1]),
                        ("dbg_ctx", [P, NDT, QCH]), ("dbg_cs", [P, 4, QCH]),
                        ("dbg_ctxT", [P, NDT, QCH]), ("dbg_yT", [P, NDT, QCH]),
                        ("dbg_hT", [P, NDT, QCH]), ("dbg_ex", [P, QCH])]:
            dbg[nm] = nc.dram_tensor(nm, shp, dt.float32, kind="ExternalOutput")

    xb3 = xb.rearrange("(c t p) d -> c t p d", t=QT_T, p=P)  # chunk, toktile, p, d
    WqT = Wq.rearrange("(ko p) d -> p ko d", p=P)  # d_in on partitions
    WkT = Wk.rearrange("(ko p) d -> p ko d", p=P)
    WvT = Wv.rearrange("(ko p) d -> p ko d", p=P)
    WoT = Wo.rearrange("(ko p) d -> p ko d", p=P)
    W1T = W1.rearrange("(ko p) f -> p ko f", p=P)
    W2T = W2.rearrange("(ko p) d -> p ko d", p=P)

    with tile.TileContext(nc) as tc:
        with (
            tc.tile_pool(name="consts", bufs=1) as consts,
            tc.tile_pool(name="mid", bufs=4) as mid,
            tc.tile_pool(name="ktb", bufs=1) as ktb_pool,
            tc.tile_pool(name="vb", bufs=1) as vb_pool,
            tc.tile_pool(name="wraw", bufs=2) as wraw,
            tc.tile_pool(name="wrnd", bufs=2) as wrnd,
            tc.tile_pool(name="expp", bufs=3) as expp,
            tc.tile_pool(name="small", bufs=2) as small,
            tc.tile_pool(name="small1", bufs=1) as small1,
            tc.tile_pool(name="ffs", bufs=2) as ffs,
            tc.tile_pool(name="dram", bufs=1, space="DRAM") as dram,
            tc.tile_pool(name="ps_a", bufs=4, space="PSUM") as ps_a,
            tc.tile_pool(name="ps_sc", bufs=2, space="PSUM") as ps_sc,
            tc.tile_pool(name="ps_ctx", bufs=2, space="PSUM") as ps_ctx,
        ):
            # ---- constants ----
            ident = consts.tile([P, P], dt.float32, tag="ident")
            make_identity(nc, ident[:])
            scr32 = consts.tile([P, QCH], dt.float32, tag="scr32")
            ones_r = consts.tile([P, P], MMDT, tag="ones")
            nc.vector.memset(scr32[:], 1.0)
            nc.vector.tensor_copy(out=ones_r[:], in_=scr32[:, 0:P])
            ones64 = consts.tile([P, HD], MMDT, tag="ones64")
            nc.vector.tensor_copy(out=ones64[:], in_=scr32[:, 0:HD])
            tri = consts.tile([P, QT_T, QCH], MMDT, tag="tri")
            for j in range(QT_T):
                # keep where f - p - 128j >= 0  <=>  (128j + p) <= f
                nc.vector.memset(scr32[:], 1.0)
                nc.gpsimd.affine_select(
                    out=scr32[:], in_=scr32[:],
                    compare_op=OP.is_ge, fill=0.0,
                    base=-P * j, pattern=[[1, QCH]], channel_multiplier=-1,
                )
                nc.vector.tensor_copy(out=tri[:, j, :], in_=scr32[:])
            kbias_sb = consts.tile([P, KT_TILES], dt.float32, tag="kbias")
            nc.sync.dma_start(kbias_sb[:], kbias[:])
            eps_sb = consts.tile([P, 1], dt.float32, tag="eps")
            nc.vector.memset(eps_sb[:], EPS)

            def load_vec_pd(name, ap, n):
                t = consts.tile([P, n], dt.float32, tag=name)
                nc.sync.dma_start(t[:], ap.rearrange("(o p) -> p o", p=P))
                return t

            bq_sb = load_vec_pd("bq", bq, NDT)
            bk_sb = load_vec_pd("bk", bk, NDT)
            bo_sb = load_vec_pd("bo", bo, NDT)
            b1_sb = load_vec_pd("b1", b1, NFT)
            b2_sb = load_vec_pd("b2", b2, NDT)
            g1_sb = load_vec_pd("g1", g1, NDT)
            be1_sb = load_vec_pd("be1", be1, NDT)
            g2_sb = load_vec_pd("g2", g2, NDT)
            be2_sb = load_vec_pd("be2", be2, NDT)
            bv_sb = consts.tile([P, D], dt.float32, tag="bv")
            nc.gpsimd.dma_start(out=bv_sb[:], in_=bv[None, :].to_broadcast([P, D]))
            colsum = consts.tile([P, 4, QCH], MMDT, tag="colsum")
            nc.vector.memset(scr32[:], 0.0)
            for _s in range(4):
                nc.vector.tensor_copy(out=colsum[:, _s, :], in_=scr32[:])

            ff_dram = dram.tile([NFT, P, QCH], MMDT)

            def transpose_in(src_ap, dst_tile, dst_do, dst_cols):
                """dst[:, dst_do, dst_cols] = (128x128 fp32 block).T via PE."""
                pt = ps_sc.tile([P, P], dt.float32, tag="ps_sc")
                nc.tensor.transpose(pt[:], src_ap, ident[:])
                nc.vector.tensor_copy(out=dst_tile[:, dst_do, dst_cols], in_=pt[:])

            def stream_round(dram_ap, shape, tag):
                """DMA a weight tile and round fp32 -> f32r on the Scalar
engine."""
                raw = wraw.tile(shape, dt.float32, tag=tag)
                nc.sync.dma_start(raw[:], dram_ap)
                rnd = wrnd.tile(shape, MMDT, tag=tag + "_r")
                nc.vector.tensor_copy(out=rnd[:], in_=raw[:])
                return rnd

            # ---- fused K/V projection + attention, chunk by chunk ----
            # (rotated order: chunk 0 IS the query chunk -> Q projected there)
            QT = mid.tile([P, NDT, QCH], MMDT, tag="mid", name="QT")
            xqT = mid.tile([P, NDT, QCH], MMDT, tag="mid", name="xqT")
            ctx_sb = mid.tile([P, NDT, QCH], dt.float32, tag="mid", name="ctx_sb")
            nc.vector.memset(ctx_sb[:], 0.0)
            for c in range(NCH):
                xTc = mid.tile([P, NDT, QCH], MMDT, tag="mid", name="xTc")
                for t in range(QT_T):
                    xn = small.tile([P, D], dt.float32, tag="xnat")
                    nc.sync.dma_start(xn[:], xb3[c, t])
                    for do in range(NDT):
                        transpose_in(xn[:, do * P:(do + 1) * P], xTc, do,
                                     slice(t * P, (t + 1) * P))
                if c == 0:
                    # query chunk: keep a copy for the residual, project Q
                    nc.vector.tensor_copy(out=xqT[:], in_=xTc[:])
                    for do in range(NDT):
                        wq_r = stream_round(WqT[:, :, do * P:(do + 1) * P],
                                            [P, NDT, P], "wsm")
                        pq = ps_a.tile([P, QCH], dt.float32, tag="ps_a")
                        for k in range(NDT):
                            nc.tensor.matmul(pq[:], wq_r[:, k, :], xTc[:, k, :],
                                             start=(k == 0), stop=(k == NDT - 1))
                        nc.vector.tensor_scalar(
                            out=QT[:, do, :], in0=pq[:],
                            scalar1=bq_sb[:, do:do + 1], scalar2=None, op0=OP.add)
                # K block: [d_out, 512 k-tokens]
                ktblk = ktb_pool.tile([P, NDT, QCH], MMDT, tag="ktb")
                for do in range(NDT):
                    wk_r = stream_round(WkT[:, :, do * P:(do + 1) * P],
                                        [P, NDT, P], "wsm")
                    pk = ps_a.tile([P, QCH], dt.float32, tag="ps_a")
                    for k in range(NDT):
                        nc.tensor.matmul(pk[:], wk_r[:, k, :], xTc[:, k, :],
                                         start=(k == 0), stop=(k == NDT - 1))
                    nc.vector.tensor_scalar(
                        out=ktblk[:, do, :], in0=pk[:],
                        scalar1=bk_sb[:, do:do + 1], scalar2=None, op0=OP.add)
                # V block: [tok, head, 64+1] with ones column
                vblk = vb_pool.tile([P, QT_T, H, HD + 1], MMDT, tag="vb")
                nc.vector.tensor_copy(out=vblk[:, :, :, HD], in_=ones64[:])
                for nh in range(2):
                    pvs = [ps_a.tile([P, QCH], dt.float32, tag="ps_a",
                                     name=f"pv{t}") for t in range(QT_T)]
                    for k in range(NDT):
                        wv_r = stream_round(
                            WvT[:, k, nh * QCH:(nh + 1) * QCH],
                            [P, QCH], "wv")
                        for t in range(QT_T):
                            nc.tensor.matmul(
                                pvs[t][:], xTc[:, k, t * P:(t + 1) * P],
                                wv_r[:],
                                start=(k == 0), stop=(k == NDT - 1))
                    for t in range(QT_T):
                        nc.vector.tensor_tensor(
                            vblk[:, t, nh * 8:(nh + 1) * 8, 0:HD],
                            pvs[t][:].rearrange("p (h d) -> p h d", d=HD),
                            bv_sb[:, nh * QCH:(nh + 1) * QCH].rearrange(
                                "p (h d) -> p h d", d=HD),
                            OP.add)
                if debug and c == 0:
                    nc.sync.dma_start(dbg["dbg_kt"][:], ktblk[:].bitcast(dt.float32))
                    nc.sync.dma_start(dbg["dbg_v"][:], vblk[:].bitcast(dt.float32))
                # attention: head pairs share a d-tile; the two K=64 score
                # matmuls go to disjoint PE row-strips (0-63 / 64-127) and
                # run concurrently via tile_position.
                for a in range(H // 2):
                    pcs = [ps_ctx.tile([P, QCH], dt.float32, tag="ps_ctx",
                                       name=f"pc{i}") for i in range(2)]
                    for j in range(QT_T):
                        ktg = c * QT_T + j
                        exs = []
                        for i in range(2):
                            bp = i * HD
                            psc = ps_sc.tile([P, QCH], dt.float32, tag="ps_sc",
                                             name=f"psc{i}")
                            nc.tensor.matmul(
                                psc[:], ktblk[bp:bp + HD, a, j * P:(j + 1) * P],
                                QT[bp:bp + HD, a, :], start=True, stop=True,
                                tile_position=(bp, 0))
                            ex = expp.tile([P, QCH], MMDT, tag="exp",
                                           name=f"ex{i}")
                            nc.scalar.activation(
                                out=ex[:], in_=psc[:], func=AF.Exp,
                                bias=kbias_sb[:, ktg:ktg + 1], scale=0.125)
                            if c == 0:
                                nc.vector.tensor_tensor(ex[:], ex[:],
                                                        tri[:, j, :], OP.mult)
                            exs.append(ex)
                        if debug and c == 0 and a == 0 and j == 0:
                            nc.sync.dma_start(dbg["dbg_ex"][:],
                                              exs[0][:].bitcast(dt.float32))
                        for i in range(2):
                            h = 2 * a + i
                            nc.tensor.matmul(
                                pcs[i][0:HD + 1, :], vblk[:, j, h, :], exs[i][:],
                                start=(j == 0), stop=(j == QT_T - 1))
                    for i in range(2):
                        h = 2 * a + i
                        bp = i * HD
                        nc.vector.tensor_tensor(
                            ctx_sb[bp:bp + HD, a, :], ctx_sb[bp:bp + HD, a, :],
                            pcs[i][0:HD, :], OP.add)
                        cb, cs = 32 * (h % 4), h // 4
                        nc.vector.tensor_tensor(
                            colsum[cb:cb + 1, cs, :], colsum[cb:cb + 1, cs, :],
                            pcs[i][HD:HD + 1, :], OP.add)

            if debug:
                nc.sync.dma_start(dbg["dbg_xqT"][:], xqT[:].bitcast(dt.float32))
                nc.sync.dma_start(dbg["dbg_QT"][:], QT[:].bitcast(dt.float32))
                nc.sync.dma_start(dbg["dbg_ctx"][:], ctx_sb[:])
                nc.sync.dma_start(dbg["dbg_cs"][:], colsum[:].bitcast(dt.float32))
            # normalize context -> f32r: reciprocal colsum, then broadcast
            # each head's row across partitions via a K=1 ones matmul in PSUM
            with nc.allow_low_precision(reason="f32r recip colsum, ~1e-4 ok"):
                for _s in range(4):
                    nc.vector.reciprocal(out=colsum[:, _s, :],
                                         in_=colsum[:, _s, :])
            ctxT = mid.tile([P, NDT, QCH], MMDT, tag="mid", name="ctxT")
            for h in range(H):
                dti, bp = h // 2, (h % 2) * HD
                cb, cs = 32 * (h % 4), h // 4
                prc = ps_sc.tile([P, QCH], dt.float32, tag="ps_sc")
                nc.tensor.matmul(prc[:], ones_r[cb:cb + 1, :],
                                 colsum[cb:cb + 1, cs, :], start=True, stop=True,
                                 tile_position=(cb, 0))
                nc.vector.tensor_tensor(
                    ctxT[bp:bp + HD, dti, :], ctx_sb[bp:bp + HD, dti, :],
                    prc[bp:bp + HD, :], OP.mult)

            # ---- O-proj + residual + LN1 ----
            yT = mid.tile([P, NDT, QCH], MMDT, tag="mid", name="yT")
            for do in range(NDT):
                wo_r = stream_round(WoT[:, :, do * P:(do + 1) * P],
                                    [P, NDT, P], "wsm")
                po = ps_a.tile([P, QCH], dt.float32, tag="ps_a")
                for k in range(NDT):
                    nc.tensor.matmul(po[:], wo_r[:, k, :], ctxT[:, k, :],
                                     start=(k == 0), stop=(k == NDT - 1))
                nc.vector.scalar_tensor_tensor(
                    out=yT[:, do, :], in0=po[:], scalar=bo_sb[:, do:do + 1],
                    in1=xqT[:, do, :], op0=OP.add, op1=OP.add)

            def layer_norm(src, dst, g_sb, be_sb):
                """dst[:, do, :] = LN(src) over d_model (partition + do axes);
                per-token (free-axis) stats via ones-matmul column sums."""
                ps1 = ps_a.tile([P, QCH], dt.float32, tag="ps_a")
                for do in range(NDT):
                    nc.tensor.matmul(ps1[:], ones_r[:], src[:, do, :],
                                     start=(do == 0), stop=(do == NDT - 1))
                ps2 = ps_a.tile([P, QCH], dt.float32, tag="ps_a")
                for do in range(NDT):
                    sq = small1.tile([P, QCH], MMDT, tag="sq")
                    nc.vector.tensor_tensor(sq[:], src[:, do, :], src[:, do, :],
                                            OP.mult)
                    nc.tensor.matmul(ps2[:], ones_r[:], sq[:],
                                     start=(do == 0), stop=(do == NDT - 1))
                mean = small1.tile([P, QCH], MMDT, tag="mean")
                nc.vector.tensor_scalar(out=mean[:], in0=ps1[:], scalar1=1.0 / D,
                                        scalar2=None, op0=OP.mult)
                m2 = small1.tile([P, QCH], MMDT, tag="m2")
                nc.vector.tensor_tensor(m2[:], mean[:], mean[:], OP.mult)
                var = small1.tile([P, QCH], MMDT, tag="var")
                nc.vector.scalar_tensor_tensor(
                    out=var[:], in0=ps2[:], scalar=1.0 / D, in1=m2[:],
                    op0=OP.mult, op1=OP.subtract)
                sstd = small1.tile([P, QCH], MMDT, tag="sstd")
                nc.scalar.activation(out=sstd[:], in_=var[:], func=AF.Sqrt,
                                     bias=eps_sb[:], scale=1.0)
                rstd = small1.tile([P, QCH], MMDT, tag="rstd")
                with nc.allow_low_precision(reason="f32r rstd, ~1e-4 rel ok"):
                    nc.vector.reciprocal(out=rstd[:], in_=sstd[:])
                for do in range(NDT):
                    t1 = small.tile([P, QCH], MMDT, tag="ln_t1")
                    nc.vector.tensor_tensor(t1[:], src[:, do, :], mean[:],
                                            OP.subtract)
                    nc.vector.tensor_tensor(t1[:], t1[:], rstd[:], OP.mult)
                    nc.vector.tensor_scalar(
                        out=dst[:, do, :], in0=t1[:],
                        scalar1=g_sb[:, do:do + 1], scalar2=be_sb[:, do:do + 1],
                        op0=OP.mult, op1=OP.add)

            hT = mid.tile([P, NDT, QCH], MMDT, tag="mid", name="hT")
            layer_norm(yT, hT, g1_sb, be1_sb)
            if debug:
                nc.sync.dma_start(dbg["dbg_ctxT"][:], ctxT[:].bitcast(dt.float32))
                nc.sync.dma_start(dbg["dbg_yT"][:], yT[:].bitcast(dt.float32))
                nc.sync.dma_start(dbg["dbg_hT"][:], hT[:].bitcast(dt.float32))

            # ---- FFN (d_ff intermediate bounces through DRAM) ----
            for ft in range(NFT):
                w1_r = stream_round(W1T[:, :, ft * P:(ft + 1) * P],
                                    [P, NDT, P], "wsm")
                pf = ps_a.tile([P, QCH], dt.float32, tag="ps_a")
                for k in range(NDT):
                    nc.tensor.matmul(pf[:], w1_r[:, k, :], hT[:, k, :],
                                     start=(k == 0), stop=(k == NDT - 1))
                ffo = ffs.tile([P, QCH], MMDT, tag="ffo")
                nc.scalar.activation(out=ffo[:], in_=pf[:], func=AF.Relu,
                                     bias=b1_sb[:, ft:ft + 1], scale=1.0)
                nc.sync.dma_start(ff_dram[ft], ffo[:])
            y2T = mid.tile([P, NDT, QCH], MMDT, tag="mid", name="y2T")
            for dog in range(2):
                pds = [(ps_a if d4 < 2 else ps_ctx).tile(
                    [P, QCH], dt.float32, tag=("ps_a" if d4 < 2 else "ps_ctx"),
                    name=f"pd{d4}") for d4 in range(4)]
                for k in range(NFT):
                    ffi = ffs.tile([P, QCH], MMDT, tag="ffi")
                    nc.sync.dma_start(ffi[:], ff_dram[k])
                    w2_r = stream_round(W2T[:, k, dog * QCH:(dog + 1) * QCH],
                                        [P, QCH], "wv")
                    for d4 in range(4):
                        nc.tensor.matmul(
                            pds[d4][:], w2_r[:, d4 * P:(d4 + 1) * P], ffi[:],
                            start=(k == 0), stop=(k == NFT - 1))
                for d4 in range(4):
                    do = dog * 4 + d4
                    nc.vector.scalar_tensor_tensor(
                        out=y2T[:, do, :], in0=pds[d4][:],
                        scalar=b2_sb[:, do:do + 1], in1=hT[:, do, :],
                        op0=OP.add, op1=OP.add)

            outT = mid.tile([P, NDT, QCH], dt.float32, tag="mid", name="outT")
            layer_norm(y2T, outT, g2_sb, be2_sb)

            # ---- transpose back, DMA out ----
            out3 = out.rearrange("(t p) d -> t p d", p=P)
            for t in range(QT_T):
                on = small.tile([P, D], dt.float32, tag="xnat")
                for do in range(NDT):
                    pt = ps_sc.tile([P, P], dt.float32, tag="ps_sc")
                    nc.tensor.transpose(pt[:], outT[:, do, t * P:(t + 1) * P],
                                        ident[:])
                    nc.vector.tensor_copy(out=on[:, do * P:(do + 1) * P], in_=pt[:])
                nc.sync.dma_start(out3[t], on[:])

    nc.finalize()
    return nc


def _get_nc(mm_dtype_name="float32r", debug=False):
    key = ("nc", mm_dtype_name, debug)
    if key not in _CACHE:
        _CACHE[key] = _build(mm_dtype_name, debug)
    return _CACHE[key]


def kernel(x, mask, Wq, bq, Wk, bk, Wv, bv, Wo, bo, W1, b1, W2, b2,
           gamma1, beta1, gamma2, beta2, _trace=False, _mm_dtype="float32r",
           _debug=False):
    from concourse.bass_utils import run_bass_kernel_spmd

    nc = _get_nc(_mm_dtype, _debug)
    x = np.ascontiguousarray(np.asarray(x, dtype=np.float32))
    shared = {
        "Wq": np.asarray(Wq, np.float32), "Wk": np.asarray(Wk, np.float32),
        "Wv": np.asarray(Wv, np.float32), "Wo": np.asarray(Wo, np.float32),
        "W1": np.asarray(W1, np.float32), "W2": np.asarray(W2, np.float32),
        "bq": np.asarray(bq, np.float32), "bk": np.asarray(bk, np.float32),
        "bv": np.asarray(bv, np.float32), "bo": np.asarray(bo, np.float32),
        "b1": np.asarray(b1, np.float32), "b2": np.asarray(b2, np.float32),
        "g1": np.asarray(gamma1, np.float32), "be1": np.asarray(beta1, np.float32),
        "g2": np.asarray(gamma2, np.float32), "be2": np.asarray(beta2, np.float32),
    }
    in_maps = []
    for c in range(NCORES):
        b, i = divmod(c, NCORES // B)
        q0 = i * QCH
        xb_rot = np.concatenate(
            [x[b, q0:q0 + QCH], x[b, :q0], x[b, q0 + QCH:]], axis=0)
        kb = np.zeros((P, KT_TILES), np.float32)
        n_ok = QT_T + q0 // P  # self tiles + past tiles
        kb[:, n_ok:] = NEG
        in_maps.append({
            "xb": np.ascontiguousarray(xb_rot),
            "kbias": kb,
            **shared,
        })
    res = run_bass_kernel_spmd(nc, in_maps, core_ids=list(range(NCORES)),
                               trace=_trace)
    out = np.empty((B, S, D), np.float32)
    for c in range(NCORES):
        b, i = divmod(c, NCORES // B)
        out[b, i * QCH:(i + 1) * QCH] = res.results[c]["out"]
    if _trace:
        _CACHE["last_result"] = res
    return out



# revision 14
# speedup vs baseline: 2.0649x; 2.0649x over previous
"""Decoder layer (attn + FFN + 2 layernorms) on 8 Trainium2 cores.

Sharding: core c handles batch b = c//4, query chunk i = c%4 (512 tokens).
Each core redundantly computes K/V for the full sequence (communication-free).
Causality: key/value token order is rotated per core on the host (self chunk
first, then past, then future) so the mask structure is uniform: k-tiles 0-3
(self chunk) get a host-built triangular multiplicative mask, other chunks
get a per-core additive bias (0 past / -1e30 future) folded into the exp.
Softmax runs unnormalized; the denominator comes from an appended ones-column
on V, accumulated at quad-aligned partitions of `colsum`, reciprocated per
slot and broadcast back over head rows with static sel-plane matmuls.

v2 vs the 976us baseline (488us):
- fp16 matmul operands everywhere (same 1 cyc/row PE rate as f32r, half the
  DMA/SBUF bytes, 2x DVE rate, true tile_position overlap of the K=64 score
  head-pairs).  PSUM / stats / context accumulation stay f32.
- Weights and x pre-tiled / pre-transposed / fp16-cast on the HOST into
  partition-major contiguous layouts (no 512B-fragmented DMA, no f32r casts,
  no PE input transposes).
- LayerNorm1 folded into FFN1 (W1' = diag(g1) W1; stats commute past the
  matmul), h materialized off-path for the residual only.
- d_ff intermediate in SBUF; FFN2 half 0 pipelined one step behind FFN1.
- Chunk c attention interleaves chunk c+1 K/V projection units so exp
  latency never starves the PE (keeps the 2.4GHz p-state).

v2.1 (this file):
- No output transposes: out is DMA'd [d, tok] and un-transposed on host.
- ctxT normalization interleaved into chunk 3's pair loop (recips + sel
  broadcasts + mults happen as each head pair finalizes).
- FFN1 psum rotation 4 deep (alternating pools) to ride out the LN1 chain;
  FFN2 half 0 skewed one ft behind FFN1.
- LN2 apply and h materialization split across Vector and GpSimd.
- Startup: x chunk 0 DMA'd per k-tile first, cold constants deferred.
"""

import sys

sys.path.insert(0, "/opt/trn_rl_repo")

import numpy as np

D = 1024          # d_model
H = 16            # heads
HD = 64           # head dim
DFF = 4096
EPS = 1e-6
B, S = 2, 2048
QCH = 512         # query tokens per core
NCORES = 8
P = 128
NCH = S // QCH            # 4 chunks of k/v tokens
NDT = D // P              # 8 d_model tiles
NFT = DFF // P            # 32 d_ff tiles
QT_T = QCH // P           # 4 query token tiles
NEG = -1.0e30

_CACHE = {}


def _build(debug=False):
    import concourse.bacc as bacc
    import concourse.mybir as mybir
    import concourse.tile as tile

    dt = mybir.dt
    F16 = dt.float16
    F32 = dt.float32
    AF = mybir.ActivationFunctionType
    OP = mybir.AluOpType

    nc = bacc.Bacc("TRN2", target_bir_lowering=False, debug=False)

    # ---- I/O (all pre-tiled on host; see _prep_shared for layouts) ----
    xt = nc.dram_tensor("xt", [NCH, P, NDT, QCH], F16, kind="ExternalInput")
    wq = nc.dram_tensor("wq", [NDT, P, NDT, P], F16, kind="ExternalInput")
    wk = nc.dram_tensor("wk", [NDT, P, NDT, P], F16, kind="ExternalInput")
    wv = nc.dram_tensor("wv", [2, NDT, P, QCH], F16, kind="ExternalInput")
    wo = nc.dram_tensor("wo", [NDT, P, NDT, P], F16, kind="ExternalInput")
    w1 = nc.dram_tensor("w1", [NFT, P, NDT, P], F16, kind="ExternalInput")
    w2 = nc.dram_tensor("w2", [2, NFT, P, QCH], F16, kind="ExternalInput")
    ones_d = nc.dram_tensor("ones", [P, P], F16, kind="ExternalInput")
    tri_d = nc.dram_tensor("tri", [P, QT_T, QCH], F16, kind="ExternalInput")
    kbias_d = nc.dram_tensor("kbias", [P, NCH * QT_T], F32,
                             kind="ExternalInput")
    selab_d = nc.dram_tensor("selab", [P, 2, P], F16, kind="ExternalInput")
    bvb_d = nc.dram_tensor("bvb", [P, D], F16, kind="ExternalInput")
    # per-partition bias/scale columns: [P, n] with vec[o*128+p] at [p, o]
    # cols8 order: bq, bk, bo, b2, g1, be1, g2, be2
    cols8_d = nc.dram_tensor("cols8", [P, 8, NDT], F32, kind="ExternalInput")
    # cols32 order: c1 (= W1^T be1 + b1), s1n (= -colsum(W1'))
    cols32_d = nc.dram_tensor("cols32", [P, 2, NFT], F32,
                              kind="ExternalInput")
    out = nc.dram_tensor("out", [NDT, P, QCH], F16, kind="ExternalOutput")

    from contextlib import ExitStack

    with tile.TileContext(nc) as tc:
        with ExitStack() as _stk:
            def pool(name, bufs, space="SBUF"):
                return _stk.enter_context(
                    tc.tile_pool(name=name, bufs=bufs, space=space))

            consts = pool("consts", 1)
            xc0p = pool("xc0p", 1)
            xcsp = pool("xcsp", 2)      # x chunks; last buf reused for ctxT
            qtp = pool("qtp", 1)
            ktp = pool("ktp", 2)
            vbp = pool("vbp", 2)
            expp = pool("expp", 2)
            ctxp = pool("ctxp", 1)
            wst = pool("wst", 4)        # streamed [P,8,P] weights
            wmv = pool("wmv", 4)        # streamed [P,512] weights
            wvp = pool("wvp", 8)        # held wv tiles
            yp = pool("yp", 1)          # yT then outT
            hp = pool("hp", 1)
            y2p = pool("y2p", 1)
            ffp = pool("ffp", 1)
            lns = pool("lns", 1)        # LN stats
            sm = pool("sm", 3)          # small scratch
            ps_q = pool("ps_q", 2, space="PSUM")
            ps_sc = pool("ps_sc", 2, space="PSUM")
            ps_ctx = pool("ps_ctx", 2, space="PSUM")

            # ---- x chunk 0 first (critical path), then warm constants ----
            xc0 = xc0p.tile([P, NDT, QCH], F16, tag="xc0")
            for k in range(NDT):
                nc.sync.dma_start(xc0[:, k, :], xt[0, :, k, :])
            cols8 = consts.tile([P, 8, NDT], F32, tag="cols8")
            nc.sync.dma_start(cols8[:], cols8_d[:])
            kbias_sb = consts.tile([P, NCH * QT_T], F32, tag="kbias")
            nc.sync.dma_start(kbias_sb[:], kbias_d[:])
            tri = consts.tile([P, QT_T, QCH], F16, tag="tri")
            nc.sync.dma_start(tri[:], tri_d[:])
            ones16 = consts.tile([P, P], F16, tag="ones")
            nc.sync.dma_start(ones16[:], ones_d[:])
            bv_sb = consts.tile([P, D], F16, tag="bvb")
            nc.sync.dma_start(bv_sb[:], bvb_d[:])
            bq_c, bk_c, bo_c, b2_c = [cols8[:, j] for j in range(4)]
            g1_c, be1_c, g2_c, be2_c = [cols8[:, 4 + j] for j in range(4)]

            def stream_w8(dram_ap):
                t = wst.tile([P, NDT, P], F16, tag="w8")
                nc.sync.dma_start(t[:], dram_ap)
                return t

            def stream_w512(dram_ap):
                t = wmv.tile([P, QCH], F16, tag="w512")
                nc.sync.dma_start(t[:], dram_ap)
                return t

            # ---- Q projection ----
            QT = qtp.tile([P, NDT, QCH], F16, tag="qt", name="QT")
            for do in range(NDT):
                wq_t = stream_w8(wq[do])
                pq = ps_q.tile([P, QCH], F32, tag="ps_q")
                for k in range(NDT):
                    nc.tensor.matmul(pq[:], wq_t[:, k, :], xc0[:, k, :],
                                     start=(k == 0), stop=(k == NDT - 1))
                nc.vector.tensor_scalar(
                    out=QT[:, do, :], in0=pq[:],
                    scalar1=bq_c[:, do:do + 1], scalar2=None, op0=OP.add)

            # ---- cold constants (not needed until late phases) ----
            selab = consts.tile([P, 2, P], F16, tag="selab")
            nc.sync.dma_start(selab[:], selab_d[:])
            cols32 = consts.tile([P, 2, NFT], F32, tag="cols32")
            nc.sync.dma_start(cols32[:], cols32_d[:])
            c1_c = cols32[:, 0]
            s1n_c = cols32[:, 1]
            eps_sb = consts.tile([P, 1], F32, tag="eps")
            nc.vector.memset(eps_sb[:], EPS)

            # ---- projection unit emitters ----
            def emit_k_tile(xin, ktblk, do):
                wk_t = stream_w8(wk[do])
                pk = ps_q.tile([P, QCH], F32, tag="ps_q")
                for k in range(NDT):
                    nc.tensor.matmul(pk[:], wk_t[:, k, :], xin[:, k, :],
                                     start=(k == 0), stop=(k == NDT - 1))
                nc.vector.tensor_scalar(
                    out=ktblk[:, do, :], in0=pk[:],
                    scalar1=bk_c[:, do:do + 1], scalar2=None, op0=OP.add)

            def load_wv_tiles(nh):
                tiles = []
                for k in range(NDT):
                    t = wvp.tile([P, QCH], F16, tag="wv")
                    nc.sync.dma_start(t[:], wv[nh, k])
                    tiles.append(t)
                return tiles

            def emit_v_t(xin, vblk, nh, t, wv_tiles):
                pv = ps_q.tile([P, QCH], F32, tag="ps_q")
                for k in range(NDT):
                    nc.tensor.matmul(
                        pv[:], xin[:, k, t * P:(t + 1) * P], wv_tiles[k][:],
                        start=(k == 0), stop=(k == NDT - 1))
                nc.vector.tensor_tensor(
                    vblk[:, t, nh * 8:(nh + 1) * 8, 0:HD],
                    pv[:].rearrange("p (h d) -> p h d", d=HD),
                    bv_sb[:, nh * QCH:(nh + 1) * QCH].rearrange(
                        "p (h d) -> p h d", d=HD),
                    OP.add)

            def new_vblk():
                vblk = vbp.tile([P, QT_T, H, HD + 1], F16, tag="vb")
                nc.vector.tensor_copy(out=vblk[:, :, :, HD],
                                      in_=ones16[:, 0:HD])
                return vblk

            def proj_units(xin, ktblk, vblk):
                """16 PE-dense closures (~8 matmuls each) projecting K/V."""
                units = []
                for do in range(NDT):
                    units.append(lambda do=do: emit_k_tile(xin, ktblk, do))
                wvh = {}
                for nh in range(2):
                    def first(nh=nh):
                        wvh[nh] = load_wv_tiles(nh)
                        emit_v_t(xin, vblk, nh, 0, wvh[nh])
                    units.append(first)
                    for t in range(1, QT_T):
                        units.append(lambda nh=nh, t=t: emit_v_t(
                            xin, vblk, nh, t, wvh[nh]))
                return units

            # ---- context accumulators ----
            ctx64 = [ctxp.tile([HD, NDT, QCH], F32, tag=f"cx{i}",
                               name=f"cx{i}") for i in range(2)]
            colsum = ctxp.tile([P, NCH, QCH], F16, tag="cs")
            crec16 = ctxp.tile([P, NCH, QCH], F16, tag="crec")
            # unused partitions must stay finite: recip runs over all 128
            # rows and the sel matmul later contracts 0 * crec16 over them
            nc.vector.memset(colsum[:], 1.0)

            ctxT_holder = [None]

            def emit_ctxT(do):
                """Broadcast recips for pair `do` and normalize its ctx."""
                ctxT = ctxT_holder[0]
                prc = ps_q.tile([P, QCH], F32, tag="ps_q", name="prc")
                nc.tensor.matmul(prc[:], selab[:, do % 2, :],
                                 crec16[:, do // 2, :],
                                 start=True, stop=True)
                nc.vector.tensor_tensor(
                    ctxT[0:HD, do, :], ctx64[0][:, do, :],
                    prc[0:HD, :], OP.mult)
                nc.vector.tensor_tensor(
                    ctxT[HD:P, do, :], ctx64[1][:, do, :],
                    prc[HD:P, :], OP.mult)

            def emit_attention_pair(c, a, ktblk, vblk, fillers):
                """Scores + exp + context for head pair a of chunk c.

                fillers: closures emitting dense PE work (next-chunk K/V
                projections) popped between score and context groups to
                cover the exp latency.
                """
                pcs = [ps_ctx.tile([P, QCH], F32, tag="ps_ctx",
                                   name=f"pc{i}") for i in range(2)]
                for jg in range(2):            # j groups {0,1}, {2,3}
                    exs = []
                    for i in range(2):
                        bp = i * HD
                        psc = ps_sc.tile([P, 2, QCH], F32, tag="ps_sc",
                                         name=f"psc{i}")
                        for jj in range(2):
                            j = 2 * jg + jj
                            nc.tensor.matmul(
                                psc[:, jj, :],
                                ktblk[bp:bp + HD, a, j * P:(j + 1) * P],
                                QT[bp:bp + HD, a, :], start=True, stop=True,
                                tile_position=(bp, 0))
                        ktg = c * QT_T + 2 * jg
                        ex = expp.tile([P, 2, QCH], F16, tag="exp",
                                       name=f"ex{i}")
                        nc.scalar.activation(
                            out=ex[:], in_=psc[:], func=AF.Exp,
                            bias=kbias_sb[:, ktg:ktg + 1], scale=0.125)
                        if c == 0:
                            nc.vector.tensor_tensor(
                                ex[:], ex[:], tri[:, 2 * jg:2 * jg + 2, :],
                                OP.mult)
                        exs.append(ex)
                    if fillers:
                        fillers.pop(0)()
                    for i in range(2):
                        h = 2 * a + i
                        for jj in range(2):
                            j = 2 * jg + jj
                            nc.tensor.matmul(
                                pcs[i][0:HD + 1, :], vblk[:, j, h, :],
                                exs[i][:, jj, :],
                                start=(j == 0), stop=(j == QT_T - 1))
                # accumulate ctx (64 rows) + denominator (row 64) per parity
                for i in range(2):
                    h = 2 * a + i
                    cb, cs = 32 * (h % 4), h // 4
                    if c == 0:
                        nc.vector.tensor_copy(out=ctx64[i][:, a, :],
                                              in_=pcs[i][0:HD, :])
                        nc.vector.tensor_copy(
                            out=colsum[cb:cb + 1, cs, :],
                            in_=pcs[i][HD:HD + 1, :])
                    else:
                        nc.vector.tensor_tensor(
                            ctx64[i][:, a, :], ctx64[i][:, a, :],
                            pcs[i][0:HD, :], OP.add)
                        nc.vector.tensor_tensor(
                            colsum[cb:cb + 1, cs, :],
                            colsum[cb:cb + 1, cs, :],
                            pcs[i][HD:HD + 1, :], OP.add)
                if c == NCH - 1 and a % 2 == 1:
                    # heads 4s..4s+3 (slot s = a//2) final: recip +
                    # normalize pairs a-1, a while attention continues
                    s = a // 2
                    with nc.allow_low_precision(reason="fp16 softmax recip"):
                        nc.vector.reciprocal(out=crec16[:, s, :],
                                             in_=colsum[:, s, :])
                    emit_ctxT(a - 1)
                    emit_ctxT(a)

            # ---- chunk 0 K/V (dense; nothing to interleave yet) ----
            ktblks = {0: ktp.tile([P, NDT, QCH], F16, tag="ktb",
                                  name="ktb0")}
            vblks = {0: new_vblk()}
            for u in proj_units(xc0, ktblks[0], vblks[0]):
                u()

            # ---- attention chunk pipeline ----
            xcs = {}
            for c in range(NCH):
                nxt = c + 1
                fillers = []
                if nxt < NCH:
                    xcs[nxt] = xcsp.tile([P, NDT, QCH], F16, tag="xcs",
                                         name=f"xcs{nxt}")
                    nc.sync.dma_start(xcs[nxt][:], xt[nxt])
                    ktblks[nxt] = ktp.tile([P, NDT, QCH], F16, tag="ktb",
                                           name=f"ktb{nxt}")
                    vblks[nxt] = new_vblk()
                    fillers = proj_units(xcs[nxt], ktblks[nxt], vblks[nxt])
                else:
                    # ctxT lives in the (now dead) xcs ring: same shape,
                    # and both xcs buffers are idle during the last chunk
                    ctxT_holder[0] = xcsp.tile([P, NDT, QCH], F16,
                                               tag="xcs", name="ctxT")
                for a in range(NDT):
                    emit_attention_pair(c, a, ktblks[c], vblks[c], fillers)
                for f in fillers:   # leftovers (none expected)
                    f()
            ctxT = ctxT_holder[0]

            # ---- O-proj + residual; LN1 stats interleaved per tile ----
            yT = yp.tile([P, NDT, QCH], F16, tag="y", name="yT")
            ps1 = ps_sc.tile([P, 2, QCH], F32, tag="ps_sc", name="ps12")
            for do in range(NDT):
                wo_t = stream_w8(wo[do])
                po = ps_q.tile([P, QCH], F32, tag="ps_q")
                for k in range(NDT):
                    nc.tensor.matmul(po[:], wo_t[:, k, :], ctxT[:, k, :],
                                     start=(k == 0), stop=(k == NDT - 1))
                nc.vector.scalar_tensor_tensor(
                    out=yT[:, do, :], in0=po[:], scalar=bo_c[:, do:do + 1],
                    in1=xc0[:, do, :], op0=OP.add, op1=OP.add)
                sq = sm.tile([P, QCH], F16, tag="sq")
                nc.vector.tensor_tensor(sq[:], yT[:, do, :], yT[:, do, :],
                                        OP.mult)
                nc.tensor.matmul(ps1[:, 0, :], ones16[:], yT[:, do, :],
                                 start=(do == 0), stop=(do == NDT - 1))
                nc.tensor.matmul(ps1[:, 1, :], ones16[:], sq[:],
                                 start=(do == 0), stop=(do == NDT - 1))

            # ---- LN1 stats chain (apply is folded into FFN1) ----
            mean = lns.tile([P, QCH], F32, tag="mean")
            nc.vector.tensor_scalar(out=mean[:], in0=ps1[:, 0, :],
                                    scalar1=1.0 / D, scalar2=None,
                                    op0=OP.mult)
            m2 = sm.tile([P, QCH], F16, tag="sq", name="m2")
            nc.vector.tensor_tensor(m2[:], mean[:], mean[:], OP.mult)
            var = lns.tile([P, QCH], F32, tag="var")
            nc.vector.scalar_tensor_tensor(
                out=var[:], in0=ps1[:, 1, :], scalar=1.0 / D, in1=m2[:],
                op0=OP.mult, op1=OP.subtract)
            sstd = sm.tile([P, QCH], F16, tag="sq", name="sstd")
            nc.scalar.activation(out=sstd[:], in_=var[:], func=AF.Sqrt,
                                 bias=eps_sb[:], scale=1.0)
            rstd = lns.tile([P, QCH], F32, tag="rstd")
            nc.vector.reciprocal(out=rstd[:], in_=sstd[:])
            rstd16 = lns.tile([P, QCH], F16, tag="rstd16")
            nc.vector.tensor_copy(out=rstd16[:], in_=rstd[:])
            mr = lns.tile([P, QCH], F32, tag="mr")
            nc.vector.tensor_tensor(mr[:], mean[:], rstd[:], OP.mult)

            # h (LN1 output) for the 2nd residual -- on GpSimd, off-path
            hT = hp.tile([P, NDT, QCH], F16, tag="h")
            for do in range(NDT):
                u = sm.tile([P, QCH], F16, tag="hu", name=f"u{do}")
                nc.gpsimd.tensor_tensor(u[:], yT[:, do, :], mean[:],
                                        OP.subtract)
                nc.gpsimd.tensor_tensor(u[:], u[:], rstd16[:], OP.mult)
                nc.gpsimd.tensor_scalar(
                    out=hT[:, do, :], in0=u[:],
                    scalar1=g1_c[:, do:do + 1], scalar2=be1_c[:, do:do + 1],
                    op0=OP.mult, op1=OP.add)

            # ---- FFN1 with FFN2 dog=0 skewed one ft behind ----
            ff = ffp.tile([P, NFT, QCH], F16, tag="ff")
            pds0 = [ps_sc.tile([P, 2, QCH], F32, tag="ps_sc",
                               name=f"pd0{i}") for i in range(2)]

            def emit_ffn1(ft):
                w1_t = stream_w8(w1[ft])
                pfp = ps_q if ft % 2 == 0 else ps_ctx
                pf = pfp.tile([P, QCH], F32, tag=pfp.name, name=f"pf{ft}")
                for k in range(NDT):
                    nc.tensor.matmul(pf[:], w1_t[:, k, :], yT[:, k, :],
                                     start=(k == 0), stop=(k == NDT - 1))
                A = sm.tile([P, QCH], F16, tag="sq", name=f"A{ft}")
                nc.vector.tensor_tensor(A[:], pf[:], rstd16[:], OP.mult)
                Bv = sm.tile([P, QCH], F16, tag="sq", name=f"B{ft}")
                nc.vector.scalar_tensor_tensor(
                    out=Bv[:], in0=mr[:], scalar=s1n_c[:, ft:ft + 1],
                    in1=A[:], op0=OP.mult, op1=OP.add)
                nc.scalar.activation(out=ff[:, ft, :], in_=Bv[:],
                                     func=AF.Relu,
                                     bias=c1_c[:, ft:ft + 1], scale=1.0)

            def emit_ffn2_dog0(ft):
                w2_t = stream_w512(w2[0, ft])
                for d4 in range(4):
                    nc.tensor.matmul(
                        pds0[d4 // 2][:, d4 % 2, :],
                        w2_t[:, d4 * P:(d4 + 1) * P], ff[:, ft, :],
                        start=(ft == 0), stop=(ft == NFT - 1))

            for ft in range(NFT):
                emit_ffn1(ft)
                if ft >= 1:
                    emit_ffn2_dog0(ft - 1)
            emit_ffn2_dog0(NFT - 1)

            # ---- y2 (dog=0 half) + LN2 stats started ----
            y2T = y2p.tile([P, NDT, QCH], F16, tag="y2")
            ps2b = ps_ctx.tile([P, QCH], F32, tag="ps_ctx", name="ps2a")
            ps2c = ps_ctx.tile([P, QCH], F32, tag="ps_ctx", name="ps2b")
            for d4 in range(4):
                nc.vector.scalar_tensor_tensor(
                    out=y2T[:, d4, :], in0=pds0[d4 // 2][:, d4 % 2, :],
                    scalar=b2_c[:, d4:d4 + 1], in1=hT[:, d4, :],
                    op0=OP.add, op1=OP.add)
                sq2 = sm.tile([P, QCH], F16, tag="sq", name=f"s2{d4}")
                nc.vector.tensor_tensor(sq2[:], y2T[:, d4, :],
                                        y2T[:, d4, :], OP.mult)
                nc.tensor.matmul(ps2b[:], ones16[:], y2T[:, d4, :],
                                 start=(d4 == 0), stop=False)
                nc.tensor.matmul(ps2c[:], ones16[:], sq2[:],
                                 start=(d4 == 0), stop=False)

            # ---- FFN2 dog=1 half ----
            pds1 = [ps_sc.tile([P, 2, QCH], F32, tag="ps_sc",
                               name=f"pd1{i}") for i in range(2)]
            for k in range(NFT):
                w2_t = stream_w512(w2[1, k])
                for d4 in range(4):
                    nc.tensor.matmul(
                        pds1[d4 // 2][:, d4 % 2, :],
                        w2_t[:, d4 * P:(d4 + 1) * P], ff[:, k, :],
                        start=(k == 0), stop=(k == NFT - 1))
            for d4 in range(4):
                do = 4 + d4
                nc.vector.scalar_tensor_tensor(
                    out=y2T[:, do, :], in0=pds1[d4 // 2][:, d4 % 2, :],
                    scalar=b2_c[:, do:do + 1], in1=hT[:, do, :],
                    op0=OP.add, op1=OP.add)
                sq2 = sm.tile([P, QCH], F16, tag="sq", name=f"s2{do}")
                nc.vector.tensor_tensor(sq2[:], y2T[:, do, :],
                                        y2T[:, do, :], OP.mult)
                nc.tensor.matmul(ps2b[:], ones16[:], y2T[:, do, :],
                                 start=False, stop=(d4 == 3))
                nc.tensor.matmul(ps2c[:], ones16[:], sq2[:],
                                 start=False, stop=(d4 == 3))

            # ---- LN2 chain + apply (split Vector / GpSimd) + DMA out ----
            mean2 = lns.tile([P, QCH], F32, tag="mean")
            nc.vector.tensor_scalar(out=mean2[:], in0=ps2b[:],
                                    scalar1=1.0 / D, scalar2=None,
                                    op0=OP.mult)
            m22 = sm.tile([P, QCH], F16, tag="sq", name="m22")
            nc.vector.tensor_tensor(m22[:], mean2[:], mean2[:], OP.mult)
            var2 = lns.tile([P, QCH], F32, tag="var")
            nc.vector.scalar_tensor_tensor(
                out=var2[:], in0=ps2c[:], scalar=1.0 / D, in1=m22[:],
                op0=OP.mult, op1=OP.subtract)
            sstd2 = sm.tile([P, QCH], F16, tag="sq", name="sstd2")
            nc.scalar.activation(out=sstd2[:], in_=var2[:], func=AF.Sqrt,
                                 bias=eps_sb[:], scale=1.0)
            rstd2 = lns.tile([P, QCH], F32, tag="rstd")
            nc.vector.reciprocal(out=rstd2[:], in_=sstd2[:])
            rstd216 = lns.tile([P, QCH], F16, tag="rstd16")
            nc.vector.tensor_copy(out=rstd216[:], in_=rstd2[:])
            outT = yp.tile([P, NDT, QCH], F16, tag="y", name="outT")
            for do in range(NDT):
                eng = nc.vector if do % 2 == 0 else nc.gpsimd
                u = sm.tile([P, QCH], F16, tag="hu", name=f"o{do}")
                eng.tensor_tensor(u[:], y2T[:, do, :], mean2[:],
                                  OP.subtract)
                eng.tensor_tensor(u[:], u[:], rstd216[:], OP.mult)
                eng.tensor_scalar(
                    out=outT[:, do, :], in0=u[:],
                    scalar1=g2_c[:, do:do + 1], scalar2=be2_c[:, do:do + 1],
                    op0=OP.mult, op1=OP.add)
                nc.sync.dma_start(out[do], outT[:, do, :])

    nc.finalize()
    return nc


def _get_nc(debug=False):
    key = ("nc", debug)
    if key not in _CACHE:
        _CACHE[key] = _build(debug)
    return _CACHE[key]


def _selab():
    # [r, par, c] fp16 broadcast planes: for d-tile do (parity par = do % 2),
    # prc[c, q] = crec16[row of head 2*do + (c >= 64), do // 2, q] where the
    # denominator of head h sits at partition 32 * (h % 4).
    m = np.zeros((P, 2, P), np.float16)
    m[0, 0, 0:HD] = 1.0     # even do: head 2do at row 0
    m[32, 0, HD:P] = 1.0    # even do: head 2do+1 at row 32
    m[64, 1, 0:HD] = 1.0    # odd do: head 2do at row 64
    m[96, 1, HD:P] = 1.0    # odd do: head 2do+1 at row 96
    return m


def _tri():
    # [p, j, f] = 1 if key token (128j + p) <= query token f else 0
    t = np.zeros((P, QT_T, QCH), np.float16)
    for j in range(QT_T):
        for p in range(P):
            t[p, j, 128 * j + p:] = 1.0
    return t


def _prep_shared(Wq, bq, Wk, bk, Wv, bv, Wo, bo, W1, b1, W2, b2,
                 gamma1, beta1, gamma2, beta2):
    """Host-side pre-tiling of all weights into partition-major fp16."""
    f16 = np.float16
    f32 = np.float32

    def tile8(W):  # [D, D] -> [do, p, k, 128]; lhsT for (do,k) = [:,k,:]
        return np.ascontiguousarray(
            W.reshape(NDT, P, NDT, P).transpose(2, 1, 0, 3).astype(f16))

    Wq = np.asarray(Wq, f32)
    Wk = np.asarray(Wk, f32)
    Wv = np.asarray(Wv, f32)
    Wo = np.asarray(Wo, f32)
    W1 = np.asarray(W1, f32)
    W2 = np.asarray(W2, f32)
    g1 = np.asarray(gamma1, f32)
    be1 = np.asarray(beta1, f32)
    W1p = g1[:, None] * W1                      # fold LN1 gamma
    c1 = W1.T @ be1 + np.asarray(b1, f32)       # fold LN1 beta
    s1n = -W1p.sum(axis=0)                      # -colsum(W1')

    shared = {
        "wq": tile8(Wq),
        "wk": tile8(Wk),
        "wo": tile8(Wo),
        # Wv: [nh, k, p, 512] moving tiles
        "wv": np.ascontiguousarray(
            Wv.reshape(NDT, P, 2, QCH).transpose(2, 0, 1, 3).astype(f16)),
        # W1': [ft, p, k, 128]
        "w1": np.ascontiguousarray(
            W1p.reshape(NDT, P, NFT, P).transpose(2, 1, 0, 3).astype(f16)),
        # W2: [dog, k32, p, 512]
        "w2": np.ascontiguousarray(
            W2.reshape(NFT, P, 2, QCH).transpose(2, 0, 1, 3).astype(f16)),
        "ones": np.ones((P, P), dtype=f16),
        "bvb": np.ascontiguousarray(np.broadcast_to(
            np.asarray(bv, f32).astype(f16), (P, D))),
        "selab": _selab(),
        "tri": _tri(),
    }
    cols8 = np.zeros((P, 8, NDT), f32)
    for idx, v in enumerate([bq, bk, bo, b2, g1, be1, gamma2, beta2]):
        cols8[:, idx, :] = np.asarray(v, f32).reshape(NDT, P).T
    shared["cols8"] = cols8
    cols32 = np.zeros((P, 2, NFT), f32)
    cols32[:, 0, :] = c1.reshape(NFT, P).T
    cols32[:, 1, :] = s1n.reshape(NFT, P).T
    shared["cols32"] = cols32
    return shared


def kernel(x, mask, Wq, bq, Wk, bk, Wv, bv, Wo, bo, W1, b1, W2, b2,
           gamma1, beta1, gamma2, beta2, _trace=False, _debug=False):
    from concourse.bass_utils import run_bass_kernel_spmd

    nc = _get_nc(_debug)
    x = np.ascontiguousarray(np.asarray(x, dtype=np.float32))
    shared = _prep_shared(Wq, bq, Wk, bk, Wv, bv, Wo, bo, W1, b1, W2, b2,
                          gamma1, beta1, gamma2, beta2)
    in_maps = []
    for c in range(NCORES):
        b, i = divmod(c, NCORES // B)
        q0 = i * QCH
        xb_rot = np.concatenate(
            [x[b, q0:q0 + QCH], x[b, :q0], x[b, q0 + QCH:]], axis=0)
        # pre-transpose: [chunk, p, ko, token] fp16
        xT4 = np.ascontiguousarray(
            xb_rot.T.reshape(NDT, P, NCH, QCH).transpose(2, 1, 0, 3)
            .astype(np.float16))
        kb = np.zeros((P, NCH * QT_T), np.float32)
        n_ok = QT_T + q0 // P  # self tiles + past tiles
        kb[:, n_ok:] = NEG
        in_maps.append({"xt": xT4, "kbias": kb, **shared})
    res = run_bass_kernel_spmd(nc, in_maps, core_ids=list(range(NCORES)),
                               trace=_trace)
    outp = np.empty((B, S, D), np.float32)
    for c in range(NCORES):
        b, i = divmod(c, NCORES // B)
        o = np.asarray(res.results[c]["out"], np.float32)  # [8, 128, 512]
        outp[b, i * QCH:(i + 1) * QCH] = \
            o.transpose(2, 0, 1).reshape(QCH, D)
    if _trace:
        _CACHE["last_result"] = res
    return outp
